# revision 9
# baseline (speedup 1.0000x reference)
"""Trainium2 Bass kernel for nn_MultiHeadLiftLayer (GNN edge-signal lift).

Computes, for each edge e with endpoints (src, tgt):
    out[e, k] = relu( x[src] . a_src[k]  +  x[tgt] . a_tgt[k] ),  k = 0..3

Strategy (edge-parallel across 8 NeuronCores):
  - Each core first computes the per-node projection table
    p[n, :] = [x[n] @ a_src.T | x[n] @ a_tgt.T]  (shape [NODES, 8], f32)
    with PE matmuls (node-major PSUM output, strided weight loads so the
    p-table store DMA is contiguous), and writes it to a DRAM scratch.
  - Edges are sharded 8 ways. Each core runs two indirect (gather) DMAs
    per edge tile over its shard: one pulls p_src rows for src indices,
    one pulls p_tgt rows for tgt indices (index transform 2*i / 2*i+1
    against a (2N, 4) view of the table bakes the column offset into the
    index). A DVE add + ACT relu fuse the two, and the result is DMA'd
    out contiguously.
"""

import numpy as np

import concourse.bacc as bacc
import concourse.bass as bass
import concourse.mybir as mybir
import concourse.tile as tile
from concourse.bass import IndirectOffsetOnAxis
from concourse.bass_utils import run_bass_kernel_spmd

# ---- problem constants (hardcoded per contract) ----
N_NODES = 50000
N_EDGES = 800000
F_IN = 64
K = 4
CORES = 8

# phase 1 tiling: blocks of 2048 nodes = 128 partitions x 16 node-chunks
BLK_W = 16
BLK = 128 * BLK_W            # 2048
N_BLOCKS = 13
N_HALF = N_BLOCKS * BLK      # 26624 padded half
NODES_PAD = 2 * N_HALF       # 53248 >= 50000

# phase 2 tiling: per-core edge shard laid out [128, M]
E_C = N_EDGES // CORES       # 100000
M = (E_C + 127) // 128       # 782
E_PAD = 128 * M              # 100096
M_CHUNKS = [98] * 7 + [96]   # sum = 782

F32 = mybir.dt.float32
I32 = mybir.dt.int32

_PROGRAM_CACHE = {}


def _build_program():
    nc = bacc.Bacc("TRN2")

    x2 = nc.dram_tensor("x2", [128, N_HALF], F32, kind="ExternalInput")
    a_in = nc.dram_tensor("a_in", [128, 8], F32, kind="ExternalInput")
    src_in = nc.dram_tensor("src_idx", [128, M], I32, kind="ExternalInput")
    tgt_in = nc.dram_tensor("tgt_idx", [128, M], I32, kind="ExternalInput")
    out_d = nc.dram_tensor("out", [128, 4 * M], F32, kind="ExternalOutput")
    p_d = nc.dram_tensor("p_tab", [NODES_PAD, 8], F32)

    with tile.TileContext(nc) as tc:
        with (
            tc.tile_pool(name="const", bufs=1) as cpool,
            tc.tile_pool(name="xin", bufs=3) as xpool,
            tc.tile_pool(name="ps", bufs=4, space="PSUM") as ppool,
            tc.tile_pool(name="pstage", bufs=3) as spool,
            tc.tile_pool(name="gath", bufs=4) as gpool,
            tc.tile_pool(name="rel", bufs=3) as rpool,
        ):
            # PE LDWEIGHTS supports only a single sync-wait, so PE operands
            # are staged through DVE copies: every matmul dependency then
            # rides the single DVE semaphore lane.
            a_raw = cpool.tile([128, 8], F32)
            nc.sync.dma_start(out=a_raw[:], in_=a_in[:])
            a_sb = cpool.tile([128, 8], F32)
            nc.vector.tensor_copy(out=a_sb[:], in_=a_raw[:])
            src_sb = cpool.tile([128, M], I32)
            tgt_sb = cpool.tile([128, M], I32)
            nc.sync.dma_start(out=src_sb[:], in_=src_in[:])
            nc.sync.dma_start(out=tgt_sb[:], in_=tgt_in[:])

            # ---- phase 1: p[n, :] = x[n] @ A  (node-major) ----
            store_insts = []
            for b in range(N_BLOCKS):
                xtr = xpool.tile([128, BLK], F32, tag="xtr")
                nc.sync.dma_start(out=xtr[:], in_=x2[:, b * BLK:(b + 1) * BLK])
                xt = xpool.tile([128, BLK], F32, tag="xt")
                nc.vector.tensor_copy(out=xt[:], in_=xtr[:])
                for h in (0, 1):
                    pt = ppool.tile([128, BLK_W * 8], F32)
                    for w in range(BLK_W):
                        # node n = h*N_HALF + b*BLK + 16*j + w sits in
                        # lhsT column j -> PSUM partition j
                        lhsT = xt[64 * h:64 * h + 64,
                                  w:w + BLK - BLK_W + 1:BLK_W]
                        nc.tensor.matmul(
                            out=pt[:, 8 * w:8 * w + 8],
                            lhsT=lhsT,
                            rhs=a_sb[64 * h:64 * h + 64, :],
                            start=True,
                            stop=True,
                        )
                    st = spool.tile([128, BLK_W * 8], F32)
                    nc.vector.tensor_copy(out=st[:], in_=pt[:])
                    base = h * N_HALF + b * BLK
                    dst = p_d[base:base + BLK, :].rearrange(
                        "(j w) k -> j w k", j=128
                    )
                    ins = nc.sync.dma_start(
                        out=dst, in_=st[:].rearrange("p (w k) -> p w k", k=8)
                    )
                    store_insts.append(ins.ins)

            # ---- phase 2: gather + add + relu per edge tile ----
            # join all p-table stores into one Pool-engine nop so the
            # gathers don't each carry 8 DMA-lane sync waits
            p_join = nc.engines[mybir.EngineType.Pool].nop(
                nofuse=True, hint="p_join"
            )
            for s in store_insts:
                tile.add_dep_helper(p_join.ins, s, reason="join p stores")
            # view p as (2*NODES_PAD, 4): row 2n = p_src(n), 2n+1 = p_tgt(n)
            p_view = p_d[:, :].rearrange("n (two k) -> (n two) k", two=2)
            mo = 0
            for mc in M_CHUNKS:
                ga = gpool.tile([128, 4 * mc], F32)
                gb = gpool.tile([128, 4 * mc], F32)
                # HW indirect DMA consumes one offset per dest partition-row,
                # so gather 128 rows (one per partition) per instruction.
                for m in range(mc):
                    i1 = nc.gpsimd.indirect_dma_start(
                        out=ga[:, 4 * m:4 * m + 4],
                        out_offset=None,
                        in_=p_view,
                        in_offset=IndirectOffsetOnAxis(
                            ap=src_sb[:, mo + m:mo + m + 1], axis=0
                        ),
                    )
                    i2 = nc.gpsimd.indirect_dma_start(
                        out=gb[:, 4 * m:4 * m + 4],
                        out_offset=None,
                        in_=p_view,
                        in_offset=IndirectOffsetOnAxis(
                            ap=tgt_sb[:, mo + m:mo + m + 1], axis=0
                        ),
                    )
                    if m == 0:
                        tile.add_dep_helper(
                            i1.ins, p_join.ins, reason="gather after p"
                        )
                        tile.add_dep_helper(
                            i2.ins, p_join.ins, reason="gather after p"
                        )
                sm = rpool.tile([128, 4 * mc], F32)
                nc.vector.tensor_tensor(
                    out=sm[:], in0=ga[:], in1=gb[:], op=mybir.AluOpType.add
                )
                r = rpool.tile([128, 4 * mc], F32)
                nc.scalar.activation(
                    out=r[:], in_=sm[:],
                    func=mybir.ActivationFunctionType.Relu,
                )
                nc.sync.dma_start(out=out_d[:, 4 * mo:4 * (mo + mc)], in_=r[:])
                mo += mc

    nc.compile()
    return nc


def get_program():
    if "nc" not in _PROGRAM_CACHE:
        _PROGRAM_CACHE["nc"] = _build_program()
    return _PROGRAM_CACHE["nc"]


def make_in_maps(x, edge_index, att):
    """Marshal full inputs into per-core input maps."""
    x = np.asarray(x, dtype=np.float32)
    att = np.asarray(att, dtype=np.float32)
    ei = np.asarray(edge_index).astype(np.int64)

    # x2: [128, N_HALF]; rows 0-63 = features of nodes [0, N_HALF),
    # rows 64-127 = features of nodes [N_HALF, 2*N_HALF). zero-padded.
    xT = np.zeros((F_IN, NODES_PAD), dtype=np.float32)
    xT[:, :N_NODES] = x.T
    x2 = np.concatenate([xT[:, :N_HALF], xT[:, N_HALF:]], axis=0)
    x2 = np.ascontiguousarray(x2)

    # A: [64, 8] = [a_src.T | a_tgt.T], replicated on both partition halves
    A1 = np.empty((F_IN, 2 * K), dtype=np.float32)
    A1[:, :K] = att[:, :F_IN].T
    A1[:, K:] = att[:, F_IN:].T
    A = np.concatenate([A1, A1], axis=0)

    in_maps = []
    for c in range(CORES):
        s = ei[0, c * E_C:(c + 1) * E_C]
        t = ei[1, c * E_C:(c + 1) * E_C]
        # index transform for the (2N, 4) table view
        sp = np.zeros(E_PAD, dtype=np.int32)
        tp = np.zeros(E_PAD, dtype=np.int32)
        sp[:E_C] = 2 * s
        tp[:E_C] = 2 * t + 1
        in_maps.append({
            "x2": x2,
            "a_in": A,
            "src_idx": np.ascontiguousarray(sp.reshape(128, M)),
            "tgt_idx": np.ascontiguousarray(tp.reshape(128, M)),
        })
    return in_maps


def assemble_output(results):
    outs = []
    for c in range(CORES):
        o = np.asarray(results[c]["out"]).reshape(E_PAD, K)[:E_C]
        outs.append(o)
    return np.ascontiguousarray(np.concatenate(outs, axis=0))


def kernel(x, edge_index, att):
    nc = get_program()
    in_maps = make_in_maps(x, edge_index, att)
    res = run_bass_kernel_spmd(nc, in_maps, core_ids=list(range(CORES)))
    return assemble_output(res.results)


# revision 10
# speedup vs baseline: 1.0073x; 1.0073x over previous
"""Trainium2 Bass kernel for nn_MultiHeadLiftLayer (GNN edge-signal lift).

Computes, for each edge e with endpoints (src, tgt):
    out[e, k] = relu( x[src] . a_src[k]  +  x[tgt] . a_tgt[k] ),  k = 0..3

Strategy (edge-parallel across 8 NeuronCores):
  - Each core first computes the per-node projection table
    p[n, :] = [x[n] @ a_src.T | x[n] @ a_tgt.T]  (shape [NODES, 8], f32)
    with PE matmuls (node-major PSUM output, strided weight loads so the
    p-table store DMA is contiguous), and writes it to a DRAM scratch.
  - Edges are sharded 8 ways. Each core runs two indirect (gather) DMAs
    per edge tile over its shard: one pulls p_src rows for src indices,
    one pulls p_tgt rows for tgt indices (index transform 2*i / 2*i+1
    against a (2N, 4) view of the table bakes the column offset into the
    index). A DVE add + ACT relu fuse the two, and the result is DMA'd
    out contiguously.
"""

import numpy as np

import concourse.bacc as bacc
import concourse.bass as bass
import concourse.mybir as mybir
import concourse.tile as tile
from concourse.bass import IndirectOffsetOnAxis
from concourse.bass_utils import run_bass_kernel_spmd

# ---- problem constants (hardcoded per contract) ----
N_NODES = 50000
N_EDGES = 800000
F_IN = 64
K = 4
CORES = 8

# phase 1 tiling: blocks of 2048 nodes = 128 partitions x 16 node-chunks
BLK_W = 16
BLK = 128 * BLK_W            # 2048
N_BLOCKS = 13
N_HALF = N_BLOCKS * BLK      # 26624 padded half
NODES_PAD = 2 * N_HALF       # 53248 >= 50000

# phase 2 tiling: per-core edge shard laid out [128, M]
E_C = N_EDGES // CORES       # 100000
M = (E_C + 127) // 128       # 782
E_PAD = 128 * M              # 100096
M_CHUNKS = [98] * 7 + [96]   # sum = 782

F32 = mybir.dt.float32
I32 = mybir.dt.int32

_PROGRAM_CACHE = {}


def _build_program():
    nc = bacc.Bacc("TRN2")

    x2 = nc.dram_tensor("x2", [128, N_HALF], F32, kind="ExternalInput")
    a_in = nc.dram_tensor("a_in", [128, 8], F32, kind="ExternalInput")
    src_in = nc.dram_tensor("src_idx", [128, M], I32, kind="ExternalInput")
    tgt_in = nc.dram_tensor("tgt_idx", [128, M], I32, kind="ExternalInput")
    out_d = nc.dram_tensor("out", [128, 4 * M], F32, kind="ExternalOutput")
    p_d = nc.dram_tensor("p_tab", [NODES_PAD, 8], F32)

    with tile.TileContext(nc) as tc:
        with (
            tc.tile_pool(name="const", bufs=1) as cpool,
            tc.tile_pool(name="xin", bufs=3) as xpool,
            tc.tile_pool(name="ps", bufs=4, space="PSUM") as ppool,
            tc.tile_pool(name="pstage", bufs=3) as spool,
            tc.tile_pool(name="gath", bufs=8) as gpool,
            tc.tile_pool(name="rel", bufs=6) as rpool,
        ):
            # PE LDWEIGHTS supports only a single sync-wait, so PE operands
            # are staged through DVE copies: every matmul dependency then
            # rides the single DVE semaphore lane.
            a_raw = cpool.tile([128, 8], F32)
            nc.sync.dma_start(out=a_raw[:], in_=a_in[:])
            a_sb = cpool.tile([128, 8], F32)
            nc.vector.tensor_copy(out=a_sb[:], in_=a_raw[:])
            src_sb = cpool.tile([128, M], I32)
            tgt_sb = cpool.tile([128, M], I32)
            nc.sync.dma_start(out=src_sb[:], in_=src_in[:])
            nc.sync.dma_start(out=tgt_sb[:], in_=tgt_in[:])

            # ---- phase 1: p[n, :] = x[n] @ A  (node-major) ----
            store_insts = []
            for b in range(N_BLOCKS):
                xtr = xpool.tile([128, BLK], F32, tag="xtr")
                nc.sync.dma_start(out=xtr[:], in_=x2[:, b * BLK:(b + 1) * BLK])
                xt = xpool.tile([128, BLK], F32, tag="xt")
                nc.vector.tensor_copy(out=xt[:], in_=xtr[:])
                for h in (0, 1):
                    pt = ppool.tile([128, BLK_W * 8], F32)
                    for w in range(BLK_W):
                        # node n = h*N_HALF + b*BLK + 16*j + w sits in
                        # lhsT column j -> PSUM partition j
                        lhsT = xt[64 * h:64 * h + 64,
                                  w:w + BLK - BLK_W + 1:BLK_W]
                        nc.tensor.matmul(
                            out=pt[:, 8 * w:8 * w + 8],
                            lhsT=lhsT,
                            rhs=a_sb[64 * h:64 * h + 64, :],
                            start=True,
                            stop=True,
                        )
                    st = spool.tile([128, BLK_W * 8], F32)
                    nc.vector.tensor_copy(out=st[:], in_=pt[:])
                    base = h * N_HALF + b * BLK
                    dst = p_d[base:base + BLK, :].rearrange(
                        "(j w) k -> j w k", j=128
                    )
                    ins = nc.sync.dma_start(
                        out=dst, in_=st[:].rearrange("p (w k) -> p w k", k=8)
                    )
                    store_insts.append(ins.ins)

            # ---- phase 2: gather + add + relu per edge tile ----
            # join all p-table stores into one Pool-engine nop so the
            # gathers don't each carry 8 DMA-lane sync waits
            p_join = nc.engines[mybir.EngineType.Pool].nop(
                nofuse=True, hint="p_join"
            )
            for s in store_insts:
                tile.add_dep_helper(p_join.ins, s, reason="join p stores")
            # view p as (2*NODES_PAD, 4): row 2n = p_src(n), 2n+1 = p_tgt(n)
            p_view = p_d[:, :].rearrange("n (two k) -> (n two) k", two=2)
            mo = 0
            for mc in M_CHUNKS:
                ga = gpool.tile([128, 4 * mc], F32)
                gb = gpool.tile([128, 4 * mc], F32)
                # HW indirect DMA consumes one offset per dest partition-row,
                # so gather 128 rows (one per partition) per instruction.
                for m in range(mc):
                    i1 = nc.gpsimd.indirect_dma_start(
                        out=ga[:, 4 * m:4 * m + 4],
                        out_offset=None,
                        in_=p_view,
                        in_offset=IndirectOffsetOnAxis(
                            ap=src_sb[:, mo + m:mo + m + 1], axis=0
                        ),
                    )
                    i2 = nc.gpsimd.indirect_dma_start(
                        out=gb[:, 4 * m:4 * m + 4],
                        out_offset=None,
                        in_=p_view,
                        in_offset=IndirectOffsetOnAxis(
                            ap=tgt_sb[:, mo + m:mo + m + 1], axis=0
                        ),
                    )
                    if m == 0:
                        tile.add_dep_helper(
                            i1.ins, p_join.ins, reason="gather after p"
                        )
                        tile.add_dep_helper(
                            i2.ins, p_join.ins, reason="gather after p"
                        )
                sm = rpool.tile([128, 4 * mc], F32)
                nc.vector.tensor_tensor(
                    out=sm[:], in0=ga[:], in1=gb[:], op=mybir.AluOpType.add
                )
                r = rpool.tile([128, 4 * mc], F32)
                nc.scalar.activation(
                    out=r[:], in_=sm[:],
                    func=mybir.ActivationFunctionType.Relu,
                )
                nc.sync.dma_start(out=out_d[:, 4 * mo:4 * (mo + mc)], in_=r[:])
                mo += mc

    nc.compile()
    return nc


def get_program():
    if "nc" not in _PROGRAM_CACHE:
        _PROGRAM_CACHE["nc"] = _build_program()
    return _PROGRAM_CACHE["nc"]


def make_in_maps(x, edge_index, att):
    """Marshal full inputs into per-core input maps."""
    x = np.asarray(x, dtype=np.float32)
    att = np.asarray(att, dtype=np.float32)
    ei = np.asarray(edge_index).astype(np.int64)

    # x2: [128, N_HALF]; rows 0-63 = features of nodes [0, N_HALF),
    # rows 64-127 = features of nodes [N_HALF, 2*N_HALF). zero-padded.
    xT = np.zeros((F_IN, NODES_PAD), dtype=np.float32)
    xT[:, :N_NODES] = x.T
    x2 = np.concatenate([xT[:, :N_HALF], xT[:, N_HALF:]], axis=0)
    x2 = np.ascontiguousarray(x2)

    # A: [64, 8] = [a_src.T | a_tgt.T], replicated on both partition halves
    A1 = np.empty((F_IN, 2 * K), dtype=np.float32)
    A1[:, :K] = att[:, :F_IN].T
    A1[:, K:] = att[:, F_IN:].T
    A = np.concatenate([A1, A1], axis=0)

    in_maps = []
    for c in range(CORES):
        s = ei[0, c * E_C:(c + 1) * E_C]
        t = ei[1, c * E_C:(c + 1) * E_C]
        # index transform for the (2N, 4) table view
        sp = np.zeros(E_PAD, dtype=np.int32)
        tp = np.zeros(E_PAD, dtype=np.int32)
        sp[:E_C] = 2 * s
        tp[:E_C] = 2 * t + 1
        in_maps.append({
            "x2": x2,
            "a_in": A,
            "src_idx": np.ascontiguousarray(sp.reshape(128, M)),
            "tgt_idx": np.ascontiguousarray(tp.reshape(128, M)),
        })
    return in_maps


def assemble_output(results):
    outs = []
    for c in range(CORES):
        o = np.asarray(results[c]["out"]).reshape(E_PAD, K)[:E_C]
        outs.append(o)
    return np.ascontiguousarray(np.concatenate(outs, axis=0))


def kernel(x, edge_index, att):
    nc = get_program()
    in_maps = make_in_maps(x, edge_index, att)
    res = run_bass_kernel_spmd(nc, in_maps, core_ids=list(range(CORES)))
    return assemble_output(res.results)
